# revision 22
# baseline (speedup 1.0000x reference)
"""Trainium2 Bass kernel for nn_LNon_37460704756094 (embedding_lookup).

Math (reference):
    d   = (data - mean(data)) / std(data, ddof=1) * scalei
    s   = sigmoid(d); t = tanh(d)
    theta = interp(theta_lut, s * 119)   # theta_lut = linspace(-pi, pi, 120)
    velo  = interp(velo_lut, |t| * 119)  # velo_lut  = linspace(0, 3, 120)
    val = d * exp(velo * sin(theta)) + velo * cos(theta)
    out = (val - mean(val)) / std(val, ddof=1) * scaleo

Both LUTs are affine in the index, so interpolation collapses to affine maps
of sigmoid/|tanh|.  Using tanh(y/2) = 2*sigmoid(y) - 1, theta becomes an
affine map of u' = tanh(y/2), so the scalar-engine work per element is just
    u' = tanh(y/2), t = tanh(y), sn = sin(c1*u'+c2), cs = sin(c1*u'+c2+pi/2),
    E  = exp(v_slope * |t| * sn)
i.e. 5 activation passes, of which tanh/tanh/sin/sin share ONE activation
table set (silu_and_others) and exp a second -- minimizing ACT_TABLE_LOADs.
All element-wise arithmetic runs on the DVE in fp16 (2x/4x perf modes) with
reductions fused into accum_out, so no tensor_reduce passes over the data.

I/O is fp16 (host converts): halves HBM traffic vs fp32.

Distribution: batch-sharded over 8 cores (4 batches each = [128, 32768] f16
per core, SBUF-resident).  Global mean/std for both normalizations via
per-partition accumulation -> 8-core AllReduce of [128, 2] partials ->
ones-matmul partition-reduce+broadcast.  A dummy AllReduce and a dummy Sqrt
activation issue at t=0 to pre-warm the collective path and the sqrt table
under the input DMA.
"""

import math

import numpy as np

import concourse.bacc as bacc
import concourse.bass as bass
import concourse.mybir as mybir
import concourse.tile as tile
from concourse.bass_utils import run_bass_kernel_spmd

N_CORES = 8
P = 128
B_FULL, C, H, W = 32, 64, 128, 128
PER_CORE = B_FULL // N_CORES * C * H * W          # 4,194,304
FREE = PER_CORE // P                              # 32,768
CH = 8192                                         # A/C chunk + B group size
NCH = FREE // CH                                  # 4
N_TOTAL = B_FULL * C * H * W                      # 33,554,432

AF = mybir.ActivationFunctionType
ALU = mybir.AluOpType
AX = mybir.AxisListType
F32 = mybir.dt.float32
F16 = mybir.dt.float16

LAST_RESULT = None  # BassKernelResults of the most recent run (for test.py)

_KERNEL_CACHE = {}


def _build(consts, sim_mode=False):
    """Build the SPMD Bass program.

    `consts` = (sin_scale, sin_b1, v_slope): theta = th0 + th_slope*s =
    sin_scale*u' + sin_b1 with u' = tanh(y/2).
    """
    sin_scale, sin_b1, v_slope = consts
    halfpi = math.pi / 2.0
    # cos(theta) for theta = sin_scale*u' + sin_b1 with sin_b1 ~ 0 (symmetric
    # LUT): cos is even in u', so cos = sin(pi/2 - sin_scale*|u'| - sin_b1),
    # keeping the Sin argument inside the scalar engine's valid [-pi, pi].
    assert abs(sin_b1) < 1e-5, f"theta LUT must be symmetric (got b1={sin_b1})"
    sin_b2 = halfpi - sin_b1

    nc = bacc.Bacc(None, num_devices=N_CORES)

    # Float biases for Sin are looked up in nc.const_aps (keyed (F32, val)).
    for cv in (sin_b1, sin_b2):
        if (F32, cv) not in nc.const_aps.aps:
            t = nc.alloc_sbuf_tensor(f"const-f32-{cv}", [P, 1], F32)
            nc.gpsimd.memset(t.ap(), cv)
            nc.const_aps.aps[(F32, cv)] = t.ap()
    nc.all_engine_barrier()

    data_in = nc.dram_tensor("data", [P, FREE], F16, kind="ExternalInput")
    scal_in = nc.dram_tensor("scal", [P, 2], F32, kind="ExternalInput")
    out_dram = nc.dram_tensor("out", [P, FREE], F16, kind="ExternalOutput")

    groups = [list(range(N_CORES))]

    with tile.TileContext(nc) as tc:
        with (
            tc.tile_pool(name="big", bufs=1) as bigpool,
            tc.tile_pool(name="small", bufs=1) as smallpool,
            tc.tile_pool(name="psum", bufs=1, space="PSUM") as psumpool,
            tc.tile_pool(name="dram", bufs=1, space="DRAM") as dram,
        ):
            xb = bigpool.tile([P, FREE], F16, name="xb", tag="xb")
            # two ping-pong scratch sets for the paired B groups
            bufs = [
                [
                    bigpool.tile([P, CH], F16, name=f"s{s}b{i}", tag=f"s{s}b{i}")
                    for i in range(4)
                ]
                for s in range(2)
            ]
            # statbuf cols: [0:4) sum(x) per chunk, [4:8) sum(x^2) per chunk,
            # [8:12) sum(val) per group, [12:16) sum(val^2) per group
            statbuf = smallpool.tile([P, 16], F32, name="statbuf", tag="statbuf")
            sm = smallpool.tile([P, 32], F32, name="sm", tag="sm")
            stA = smallpool.tile([P, 2], F32, name="stA", tag="stA")
            stB = smallpool.tile([P, 2], F32, name="stB", tag="stB")
            scal_all = smallpool.tile([P, 2], F32, name="scal_all", tag="scal_all")
            ones = smallpool.tile([P, P], F32, name="ones", tag="ones")
            psumA = psumpool.tile([P, 2], F32, name="psumA", tag="psumA")
            psumB = psumpool.tile([P, 2], F32, name="psumB", tag="psumB")

            cc_a_in = dram.tile([P, 2], F32, name="cc_a_in", tag="cc_a_in")
            cc_a_out = dram.tile([P, 2], F32, name="cc_a_out", tag="cc_a_out")
            cc_b_in = dram.tile([P, 2], F32, name="cc_b_in", tag="cc_b_in")
            cc_b_out = dram.tile([P, 2], F32, name="cc_b_out", tag="cc_b_out")

            nc.gpsimd.dma_start(scal_all[:], scal_in[:])
            nc.vector.memset(ones[:], 1.0)

            # ---- warm-up: load the sqrt activation table under the input
            # DMA (the set also covers Square used in phase A)
            nc.scalar.activation(sm[:, 30:31], ones[:, 0:1], AF.Sqrt)

            # ---------------- Phase A: load + input stats ----------------
            for c in range(NCH):
                sl = slice(c * CH, (c + 1) * CH)
                nc.sync.dma_start(xb[:, sl], data_in[:, sl])
                # sum(x^2): scalar Square (in every act table set) + accum
                nc.scalar.activation(
                    bufs[0][0][:], xb[:, sl], AF.Square,
                    accum_out=statbuf[:, 4 + c : 5 + c],
                )
                # sum(x): vector reduce (fp16 DVE accum_out faults the HW,
                # so reductions go through tensor_reduce / scalar accum)
                nc.vector.reduce_sum(
                    statbuf[:, c : c + 1], xb[:, sl], axis=AX.X
                )

            nc.vector.reduce_sum(stA[:, 0:1], statbuf[:, 0:4], axis=AX.X)
            nc.vector.reduce_sum(stA[:, 1:2], statbuf[:, 4:8], axis=AX.X)

            nc.gpsimd.dma_start(cc_a_in[:], stA[:])
            if sim_mode:
                nc.gpsimd.dma_start(cc_a_out[:], cc_a_in[:])
            else:
                nc.gpsimd.collective_compute(
                    "AllReduce", ALU.add, replica_groups=groups,
                    ins=[cc_a_in.opt()], outs=[cc_a_out.opt()],
                )
            nc.gpsimd.dma_start(stA[:], cc_a_out[:])
            # ones.T @ stA reduces across partitions AND broadcasts
            nc.tensor.matmul(psumA[:], ones[:], stA[:])
            nc.vector.tensor_copy(sm[:, 0:2], psumA[:])

            # a = scalei / std, b = -mean * a   (std unbiased, ddof=1)
            nc.vector.tensor_scalar_mul(sm[:, 2:3], sm[:, 0:1], 1.0 / N_TOTAL)
            nc.vector.tensor_mul(sm[:, 3:4], sm[:, 0:1], sm[:, 2:3])
            nc.vector.tensor_sub(sm[:, 4:5], sm[:, 1:2], sm[:, 3:4])
            nc.vector.tensor_scalar_mul(sm[:, 5:6], sm[:, 4:5], 1.0 / (N_TOTAL - 1))
            nc.scalar.activation(sm[:, 6:7], sm[:, 5:6], AF.Sqrt)
            nc.vector.reciprocal(sm[:, 7:8], sm[:, 6:7])
            nc.vector.tensor_mul(sm[:, 8:9], sm[:, 7:8], scal_all[:, 0:1])   # a
            nc.vector.tensor_mul(sm[:, 9:10], sm[:, 2:3], sm[:, 8:9])
            nc.vector.tensor_scalar_mul(sm[:, 10:11], sm[:, 9:10], -1.0)     # b
            nc.vector.tensor_scalar_mul(sm[:, 11:12], sm[:, 8:9], 0.5)       # a/2
            nc.vector.tensor_scalar_mul(sm[:, 12:13], sm[:, 10:11], 0.5)     # b/2
            a_ap = sm[:, 8:9]
            b_ap = sm[:, 10:11]
            ah_ap = sm[:, 11:12]
            bh_ap = sm[:, 12:13]

            # ---------------- Phase B: 4 groups, paired over 2 buffer sets
            # Scalar passes per group: tanh(y/2), tanh(y), sin, sin [one
            # table set], then exp [second set].  Pairing groups keeps each
            # table set loaded across 2 groups -> 2 loads per pair.
            for pair in range(NCH // 2):
                gs = (2 * pair, 2 * pair + 1)
                for g in gs:
                    BU, BT, BSN, BCS = bufs[g % 2]
                    sl = slice(g * CH, (g + 1) * CH)
                    nc.scalar.activation(BU[:], xb[:, sl], AF.Tanh,
                                         bias=bh_ap, scale=ah_ap)       # u'
                    nc.scalar.activation(BT[:], xb[:, sl], AF.Tanh,
                                         bias=b_ap, scale=a_ap)         # t
                    nc.scalar.activation(BSN[:], BU[:], AF.Sin,
                                         bias=sin_b1, scale=sin_scale)  # sn
                for g in gs:
                    BU, BT, BSN, BCS = bufs[g % 2]
                    nc.vector.scalar_tensor_tensor(
                        BCS[:], BU[:], -1.0, BU[:],
                        op0=ALU.mult, op1=ALU.max)                     # |u'|
                for g in gs:
                    BU, BT, BSN, BCS = bufs[g % 2]
                    nc.scalar.activation(BU[:], BCS[:], AF.Sin,
                                         bias=sin_b2, scale=-sin_scale)  # cs
                for g in gs:
                    BU, BT, BSN, BCS = bufs[g % 2]
                    # rotate buffers so no op writes a tile it also reads
                    nc.vector.scalar_tensor_tensor(
                        BCS[:], BT[:], -1.0, BT[:],
                        op0=ALU.mult, op1=ALU.max)                     # w = |t|
                    nc.vector.tensor_tensor(BT[:], BCS[:], BSN[:],
                                            op=ALU.mult)               # p''=w*sn
                    nc.vector.tensor_tensor(BSN[:], BCS[:], BU[:],
                                            op=ALU.mult)               # q''=w*cs
                for g in gs:
                    BU, BT, BSN, BCS = bufs[g % 2]
                    nc.scalar.activation(BCS[:], BT[:], AF.Exp,
                                         scale=v_slope)                 # E
                for g in gs:
                    BU, BT, BSN, BCS = bufs[g % 2]
                    sl = slice(g * CH, (g + 1) * CH)
                    nc.vector.tensor_scalar(
                        BU[:], xb[:, sl], a_ap, b_ap,
                        op0=ALU.mult, op1=ALU.add)                      # u = a*x+b
                    nc.vector.tensor_tensor(BT[:], BU[:], BCS[:],
                                            op=ALU.mult)                # r = u*E
                    nc.vector.scalar_tensor_tensor(
                        xb[:, sl], BSN[:], v_slope, BT[:],
                        op0=ALU.mult, op1=ALU.add)                      # val
                    # sum(val^2): scalar Square+accum (same table set as Exp)
                    nc.scalar.activation(
                        BU[:], xb[:, sl], AF.Square,
                        accum_out=statbuf[:, 12 + g : 13 + g],
                    )
                    nc.vector.reduce_sum(
                        statbuf[:, 8 + g : 9 + g], xb[:, sl], axis=AX.X
                    )                                                   # sum(val)

            nc.vector.reduce_sum(stB[:, 0:1], statbuf[:, 8:12], axis=AX.X)
            nc.vector.reduce_sum(stB[:, 1:2], statbuf[:, 12:16], axis=AX.X)

            nc.gpsimd.dma_start(cc_b_in[:], stB[:])
            if sim_mode:
                nc.gpsimd.dma_start(cc_b_out[:], cc_b_in[:])
            else:
                nc.gpsimd.collective_compute(
                    "AllReduce", ALU.add, replica_groups=groups,
                    ins=[cc_b_in.opt()], outs=[cc_b_out.opt()],
                )
            nc.gpsimd.dma_start(stB[:], cc_b_out[:])
            nc.tensor.matmul(psumB[:], ones[:], stB[:])
            nc.vector.tensor_copy(sm[:, 16:18], psumB[:])

            nc.vector.tensor_scalar_mul(sm[:, 18:19], sm[:, 16:17], 1.0 / N_TOTAL)
            nc.vector.tensor_mul(sm[:, 19:20], sm[:, 16:17], sm[:, 18:19])
            nc.vector.tensor_sub(sm[:, 20:21], sm[:, 17:18], sm[:, 19:20])
            nc.vector.tensor_scalar_mul(sm[:, 21:22], sm[:, 20:21], 1.0 / (N_TOTAL - 1))
            nc.scalar.activation(sm[:, 22:23], sm[:, 21:22], AF.Sqrt)
            nc.vector.reciprocal(sm[:, 23:24], sm[:, 22:23])
            nc.vector.tensor_mul(sm[:, 24:25], sm[:, 23:24], scal_all[:, 1:2])  # a2
            nc.vector.tensor_mul(sm[:, 25:26], sm[:, 18:19], sm[:, 24:25])
            nc.vector.tensor_scalar_mul(sm[:, 26:27], sm[:, 25:26], -1.0)       # b2
            a2_ap = sm[:, 24:25]
            b2_ap = sm[:, 26:27]

            # ---------------- Phase C: normalize + store -----------------
            for c in range(NCH):
                sl = slice(c * CH, (c + 1) * CH)
                o_ = bufs[c % 2][0]
                nc.vector.tensor_scalar(
                    o_[:], xb[:, sl], a2_ap, b2_ap, op0=ALU.mult, op1=ALU.add
                )
                nc.sync.dma_start(out_dram[:, sl], o_[:])

    nc.finalize()
    return nc


def kernel(data, params, scalei, scaleo):
    global LAST_RESULT
    params = np.asarray(params, dtype=np.float32)

    # Affine-LUT coefficients from the actual params input.
    th_lut = params[0, 0]
    v_lut = params[1, 0]
    npts = th_lut.shape[0]
    th0 = float(th_lut[0])
    th_slope = float(th_lut[npts - 1]) - th0
    v0 = float(v_lut[0])
    v_slope = float(v_lut[npts - 1]) - v0
    assert abs(v0) < 1e-6, f"velocity LUT must start at 0 (got {v0})"

    # theta = th0 + th_slope*s = sin_scale*u' + sin_b1, u' = tanh(y/2)
    consts = (th_slope / 2.0, th0 + th_slope / 2.0, v_slope)
    nc = _KERNEL_CACHE.get(consts)
    if nc is None:
        nc = _build(consts)
        _KERNEL_CACHE[consts] = nc

    scal = np.tile(
        np.array(
            [[float(np.asarray(scalei).reshape(-1)[0]),
              float(np.asarray(scaleo).reshape(-1)[0])]],
            dtype=np.float32,
        ),
        (P, 1),
    )

    data = np.asarray(data)
    bpc = B_FULL // N_CORES
    in_maps = []
    for i in range(N_CORES):
        shard = (
            data[i * bpc : (i + 1) * bpc]
            .reshape(P, FREE)
            .astype(np.float16)
        )
        in_maps.append({"data": shard, "scal": scal})

    res = run_bass_kernel_spmd(nc, in_maps, core_ids=list(range(N_CORES)))
    LAST_RESULT = res

    out = np.concatenate(
        [
            r["out"].astype(np.float32).reshape(bpc, C, H, W)
            for r in res.results
        ],
        axis=0,
    )
    return out


# revision 26
# speedup vs baseline: 1.2961x; 1.2961x over previous
"""Trainium2 Bass kernel for nn_LNon_37460704756094 (embedding_lookup).

Math (reference):
    d   = (data - mean(data)) / std(data, ddof=1) * scalei
    s   = sigmoid(d); t = tanh(d)
    theta = interp(theta_lut, s * 119)   # theta_lut = linspace(-pi, pi, 120)
    velo  = interp(velo_lut, |t| * 119)  # velo_lut  = linspace(0, 3, 120)
    val = d * exp(velo * sin(theta)) + velo * cos(theta)
    out = (val - mean(val)) / std(val, ddof=1) * scaleo

Both LUTs are affine in the index, so interpolation collapses to affine maps
of sigmoid/|tanh|.  Using tanh(y/2) = 2*sigmoid(y) - 1, theta becomes an
affine map of u' = tanh(y/2), so the scalar-engine work per element is just
    u' = tanh(y/2), t = tanh(y), sn = sin(c1*u'+c2), cs = sin(c1*u'+c2+pi/2),
    E  = exp(v_slope * |t| * sn)
i.e. 5 activation passes, of which tanh/tanh/sin/sin share ONE activation
table set (silu_and_others) and exp a second -- minimizing ACT_TABLE_LOADs.
All element-wise arithmetic runs on the DVE in fp16 (2x/4x perf modes) with
reductions fused into accum_out, so no tensor_reduce passes over the data.

I/O is fp16 (host converts): halves HBM traffic vs fp32.

Distribution: batch-sharded over 8 cores (4 batches each = [128, 32768] f16
per core, SBUF-resident).  Global mean/std for both normalizations via
per-partition accumulation -> 8-core AllReduce of [128, 2] partials ->
ones-matmul partition-reduce+broadcast.  A dummy AllReduce and a dummy Sqrt
activation issue at t=0 to pre-warm the collective path and the sqrt table
under the input DMA.
"""

import math

import numpy as np

import concourse.bacc as bacc
import concourse.bass as bass
import concourse.mybir as mybir
import concourse.tile as tile
from concourse.bass_utils import run_bass_kernel_spmd

N_CORES = 8
P = 128
B_FULL, C, H, W = 32, 64, 128, 128
PER_CORE = B_FULL // N_CORES * C * H * W          # 4,194,304
FREE = PER_CORE // P                              # 32,768
CH = 8192                                         # A/C chunk + B group size
NCH = FREE // CH                                  # 4
N_TOTAL = B_FULL * C * H * W                      # 33,554,432

AF = mybir.ActivationFunctionType
ALU = mybir.AluOpType
AX = mybir.AxisListType
F32 = mybir.dt.float32
F16 = mybir.dt.float16
I16 = mybir.dt.int16

LAST_RESULT = None  # BassKernelResults of the most recent run (for test.py)

_KERNEL_CACHE = {}


def _build(consts, sim_mode=False):
    """Build the SPMD Bass program.

    `consts` = (sin_scale, sin_b1, v_slope): theta = th0 + th_slope*s =
    sin_scale*u' + sin_b1 with u' = tanh(y/2).
    """
    sin_scale, sin_b1, v_slope = consts
    halfpi = math.pi / 2.0
    # cos(theta) for theta = sin_scale*u' + sin_b1 with sin_b1 ~ 0 (symmetric
    # LUT): cos is even in u', so cos = sin(pi/2 - sin_scale*|u'| - sin_b1),
    # keeping the Sin argument inside the scalar engine's valid [-pi, pi].
    assert abs(sin_b1) < 1e-5, f"theta LUT must be symmetric (got b1={sin_b1})"
    sin_b2 = halfpi - sin_b1

    nc = bacc.Bacc(None, num_devices=N_CORES)

    # Float biases for Sin are looked up in nc.const_aps (keyed (F32, val)).
    for cv in (sin_b1, sin_b2):
        if (F32, cv) not in nc.const_aps.aps:
            t = nc.alloc_sbuf_tensor(f"const-f32-{cv}", [P, 1], F32)
            nc.gpsimd.memset(t.ap(), cv)
            nc.const_aps.aps[(F32, cv)] = t.ap()
    nc.all_engine_barrier()

    data_in = nc.dram_tensor("data", [P, FREE], F16, kind="ExternalInput")
    scal_in = nc.dram_tensor("scal", [P, 2], F32, kind="ExternalInput")
    out_dram = nc.dram_tensor("out", [P, FREE], F16, kind="ExternalOutput")

    groups = [list(range(N_CORES))]

    with tile.TileContext(nc) as tc:
        with (
            tc.tile_pool(name="big", bufs=1) as bigpool,
            tc.tile_pool(name="small", bufs=1) as smallpool,
            tc.tile_pool(name="psum", bufs=1, space="PSUM") as psumpool,
            tc.tile_pool(name="dram", bufs=1, space="DRAM") as dram,
        ):
            xb = bigpool.tile([P, FREE], F16, name="xb", tag="xb")
            # two ping-pong scratch sets for the paired B groups
            bufs = [
                [
                    bigpool.tile([P, CH], F16, name=f"s{s}b{i}", tag=f"s{s}b{i}")
                    for i in range(4)
                ]
                for s in range(2)
            ]
            # statbuf cols: [0:4) sum(x) per chunk, [4:8) sum(x^2) per chunk,
            # [8:12) sum(val) per group, [12:16) sum(val^2) per group
            statbuf = smallpool.tile([P, 16], F32, name="statbuf", tag="statbuf")
            sm = smallpool.tile([P, 32], F32, name="sm", tag="sm")
            stA = smallpool.tile([P, 2], F32, name="stA", tag="stA")
            stB = smallpool.tile([P, 2], F32, name="stB", tag="stB")
            scal_all = smallpool.tile([P, 2], F32, name="scal_all", tag="scal_all")
            ones = smallpool.tile([P, P], F32, name="ones", tag="ones")
            psumA = psumpool.tile([P, 2], F32, name="psumA", tag="psumA")
            psumB = psumpool.tile([P, 2], F32, name="psumB", tag="psumB")

            cc_w_in = dram.tile([P, 2], F32, name="cc_w_in", tag="cc_w_in")
            cc_w_out = dram.tile([P, 2], F32, name="cc_w_out", tag="cc_w_out")
            cc_a_in = dram.tile([P, 2], F32, name="cc_a_in", tag="cc_a_in")
            cc_a_out = dram.tile([P, 2], F32, name="cc_a_out", tag="cc_a_out")
            cc_b_in = dram.tile([P, 2], F32, name="cc_b_in", tag="cc_b_in")
            cc_b_out = dram.tile([P, 2], F32, name="cc_b_out", tag="cc_b_out")

            nc.gpsimd.dma_start(scal_all[:], scal_in[:])
            nc.vector.memset(ones[:], 1.0)

            # ---- warm-ups under the input DMA ----
            # (a) dummy AllReduce absorbs the collective cold-start so the
            # real stats AllReduce is cheap
            nc.vector.memset(stB[:], 0.0)
            nc.gpsimd.dma_start(cc_w_in[:], stB[:])
            if sim_mode:
                nc.gpsimd.dma_start(cc_w_out[:], cc_w_in[:])
            else:
                nc.gpsimd.collective_compute(
                    "AllReduce", ALU.add, replica_groups=groups,
                    ins=[cc_w_in.opt()], outs=[cc_w_out.opt()],
                )
            # (b) sqrt activation table (set also covers Square for phase A)
            nc.scalar.activation(sm[:, 30:31], ones[:, 0:1], AF.Sqrt)

            # ---------------- Phase A: load + input stats ----------------
            for c in range(NCH):
                sl = slice(c * CH, (c + 1) * CH)
                nc.sync.dma_start(xb[:, sl], data_in[:, sl])
                # sum(x^2): scalar Square (in every act table set) + accum
                nc.scalar.activation(
                    bufs[0][0][:], xb[:, sl], AF.Square,
                    accum_out=statbuf[:, 4 + c : 5 + c],
                )
                # sum(x): vector reduce (fp16 DVE accum_out faults the HW,
                # so reductions go through tensor_reduce / scalar accum)
                nc.vector.reduce_sum(
                    statbuf[:, c : c + 1], xb[:, sl], axis=AX.X
                )

            nc.vector.reduce_sum(stA[:, 0:1], statbuf[:, 0:4], axis=AX.X)
            nc.vector.reduce_sum(stA[:, 1:2], statbuf[:, 4:8], axis=AX.X)

            nc.gpsimd.dma_start(cc_a_in[:], stA[:])
            if sim_mode:
                nc.gpsimd.dma_start(cc_a_out[:], cc_a_in[:])
            else:
                nc.gpsimd.collective_compute(
                    "AllReduce", ALU.add, replica_groups=groups,
                    ins=[cc_a_in.opt()], outs=[cc_a_out.opt()],
                )
            nc.gpsimd.dma_start(stA[:], cc_a_out[:])
            # ones.T @ stA reduces across partitions AND broadcasts
            nc.tensor.matmul(psumA[:], ones[:], stA[:])
            nc.vector.tensor_copy(sm[:, 0:2], psumA[:])

            # a = scalei / std, b = -mean * a   (std unbiased, ddof=1)
            nc.vector.tensor_scalar_mul(sm[:, 2:3], sm[:, 0:1], 1.0 / N_TOTAL)
            nc.vector.tensor_mul(sm[:, 3:4], sm[:, 0:1], sm[:, 2:3])
            nc.vector.tensor_sub(sm[:, 4:5], sm[:, 1:2], sm[:, 3:4])
            nc.vector.tensor_scalar_mul(sm[:, 5:6], sm[:, 4:5], 1.0 / (N_TOTAL - 1))
            nc.scalar.activation(sm[:, 6:7], sm[:, 5:6], AF.Sqrt)
            nc.vector.reciprocal(sm[:, 7:8], sm[:, 6:7])
            nc.vector.tensor_mul(sm[:, 8:9], sm[:, 7:8], scal_all[:, 0:1])   # a
            nc.vector.tensor_mul(sm[:, 9:10], sm[:, 2:3], sm[:, 8:9])
            nc.vector.tensor_scalar_mul(sm[:, 10:11], sm[:, 9:10], -1.0)     # b
            nc.vector.tensor_scalar_mul(sm[:, 11:12], sm[:, 8:9], 0.5)       # a/2
            nc.vector.tensor_scalar_mul(sm[:, 12:13], sm[:, 10:11], 0.5)     # b/2
            a_ap = sm[:, 8:9]
            b_ap = sm[:, 10:11]
            ah_ap = sm[:, 11:12]
            bh_ap = sm[:, 12:13]

            # ---------------- Phase B: 4 groups, paired over 2 buffer sets
            # Scalar passes per group: tanh(y/2), tanh(y), sin, sin [one
            # table set], then exp [second set].  Pairing groups keeps each
            # table set loaded across 2 groups -> 2 loads per pair.
            for pair in range(NCH // 2):
                gs = (2 * pair, 2 * pair + 1)
                for g in gs:
                    BU, BT, BSN, BCS = bufs[g % 2]
                    sl = slice(g * CH, (g + 1) * CH)
                    nc.scalar.activation(BU[:], xb[:, sl], AF.Tanh,
                                         bias=bh_ap, scale=ah_ap)       # u'
                    nc.scalar.activation(BT[:], xb[:, sl], AF.Tanh,
                                         bias=b_ap, scale=a_ap)         # t
                    nc.scalar.activation(BSN[:], BU[:], AF.Sin,
                                         bias=sin_b1, scale=sin_scale)  # sn
                for g in gs:
                    BU, BT, BSN, BCS = bufs[g % 2]
                    # |u'| in one 4x-mode op: clear the fp16 sign bit
                    nc.vector.tensor_scalar(
                        BCS[:].bitcast(I16), BU[:].bitcast(I16),
                        0x7FFF, None, op0=ALU.bitwise_and)             # |u'|
                for g in gs:
                    BU, BT, BSN, BCS = bufs[g % 2]
                    nc.scalar.activation(BU[:], BCS[:], AF.Sin,
                                         bias=sin_b2, scale=-sin_scale)  # cs
                for g in gs:
                    BU, BT, BSN, BCS = bufs[g % 2]
                    # rotate buffers so no op writes a tile it also reads
                    nc.vector.tensor_scalar(
                        BCS[:].bitcast(I16), BT[:].bitcast(I16),
                        0x7FFF, None, op0=ALU.bitwise_and)             # w = |t|
                    nc.vector.tensor_tensor(BT[:], BCS[:], BSN[:],
                                            op=ALU.mult)               # p''=w*sn
                    nc.vector.tensor_tensor(BSN[:], BCS[:], BU[:],
                                            op=ALU.mult)               # q''=w*cs
                for g in gs:
                    BU, BT, BSN, BCS = bufs[g % 2]
                    nc.scalar.activation(BCS[:], BT[:], AF.Exp,
                                         scale=v_slope)                 # E
                for g in gs:
                    BU, BT, BSN, BCS = bufs[g % 2]
                    sl = slice(g * CH, (g + 1) * CH)
                    nc.vector.tensor_scalar(
                        BU[:], xb[:, sl], a_ap, b_ap,
                        op0=ALU.mult, op1=ALU.add)                      # u = a*x+b
                    nc.vector.tensor_tensor(BT[:], BU[:], BCS[:],
                                            op=ALU.mult)                # r = u*E
                    nc.vector.tensor_scalar_mul(
                        BU[:], BSN[:], v_slope)                         # v*q''
                    nc.vector.tensor_tensor(xb[:, sl], BT[:], BU[:],
                                            op=ALU.add)                 # val
                    nc.vector.tensor_tensor(BCS[:], xb[:, sl], xb[:, sl],
                                            op=ALU.mult)                # val^2
                    nc.vector.reduce_sum(
                        statbuf[:, 12 + g : 13 + g], BCS[:], axis=AX.X
                    )                                                   # sum val^2
                    nc.vector.reduce_sum(
                        statbuf[:, 8 + g : 9 + g], xb[:, sl], axis=AX.X
                    )                                                   # sum(val)

            nc.vector.reduce_sum(stB[:, 0:1], statbuf[:, 8:12], axis=AX.X)
            nc.vector.reduce_sum(stB[:, 1:2], statbuf[:, 12:16], axis=AX.X)

            nc.gpsimd.dma_start(cc_b_in[:], stB[:])
            if sim_mode:
                nc.gpsimd.dma_start(cc_b_out[:], cc_b_in[:])
            else:
                nc.gpsimd.collective_compute(
                    "AllReduce", ALU.add, replica_groups=groups,
                    ins=[cc_b_in.opt()], outs=[cc_b_out.opt()],
                )
            nc.gpsimd.dma_start(stB[:], cc_b_out[:])
            nc.tensor.matmul(psumB[:], ones[:], stB[:])
            nc.vector.tensor_copy(sm[:, 16:18], psumB[:])

            nc.vector.tensor_scalar_mul(sm[:, 18:19], sm[:, 16:17], 1.0 / N_TOTAL)
            nc.vector.tensor_mul(sm[:, 19:20], sm[:, 16:17], sm[:, 18:19])
            nc.vector.tensor_sub(sm[:, 20:21], sm[:, 17:18], sm[:, 19:20])
            nc.vector.tensor_scalar_mul(sm[:, 21:22], sm[:, 20:21], 1.0 / (N_TOTAL - 1))
            nc.scalar.activation(sm[:, 22:23], sm[:, 21:22], AF.Sqrt)
            nc.vector.reciprocal(sm[:, 23:24], sm[:, 22:23])
            nc.vector.tensor_mul(sm[:, 24:25], sm[:, 23:24], scal_all[:, 1:2])  # a2
            nc.vector.tensor_mul(sm[:, 25:26], sm[:, 18:19], sm[:, 24:25])
            nc.vector.tensor_scalar_mul(sm[:, 26:27], sm[:, 25:26], -1.0)       # b2
            a2_ap = sm[:, 24:25]
            b2_ap = sm[:, 26:27]

            # ---------------- Phase C: normalize + store -----------------
            for c in range(NCH):
                sl = slice(c * CH, (c + 1) * CH)
                o_ = bufs[c % 2][0]
                nc.vector.tensor_scalar(
                    o_[:], xb[:, sl], a2_ap, b2_ap, op0=ALU.mult, op1=ALU.add
                )
                nc.sync.dma_start(out_dram[:, sl], o_[:])

    nc.finalize()
    return nc


def kernel(data, params, scalei, scaleo):
    global LAST_RESULT
    params = np.asarray(params, dtype=np.float32)

    # Affine-LUT coefficients from the actual params input.
    th_lut = params[0, 0]
    v_lut = params[1, 0]
    npts = th_lut.shape[0]
    th0 = float(th_lut[0])
    th_slope = float(th_lut[npts - 1]) - th0
    v0 = float(v_lut[0])
    v_slope = float(v_lut[npts - 1]) - v0
    assert abs(v0) < 1e-6, f"velocity LUT must start at 0 (got {v0})"

    # theta = th0 + th_slope*s = sin_scale*u' + sin_b1, u' = tanh(y/2)
    consts = (th_slope / 2.0, th0 + th_slope / 2.0, v_slope)
    nc = _KERNEL_CACHE.get(consts)
    if nc is None:
        nc = _build(consts)
        _KERNEL_CACHE[consts] = nc

    scal = np.tile(
        np.array(
            [[float(np.asarray(scalei).reshape(-1)[0]),
              float(np.asarray(scaleo).reshape(-1)[0])]],
            dtype=np.float32,
        ),
        (P, 1),
    )

    data = np.asarray(data)
    bpc = B_FULL // N_CORES
    in_maps = []
    for i in range(N_CORES):
        shard = (
            data[i * bpc : (i + 1) * bpc]
            .reshape(P, FREE)
            .astype(np.float16)
        )
        in_maps.append({"data": shard, "scal": scal})

    res = run_bass_kernel_spmd(nc, in_maps, core_ids=list(range(N_CORES)))
    LAST_RESULT = res

    out = np.concatenate(
        [
            r["out"].astype(np.float32).reshape(bpc, C, H, W)
            for r in res.results
        ],
        axis=0,
    )
    return out


# revision 28
# speedup vs baseline: 1.5559x; 1.2004x over previous
"""Trainium2 Bass kernel for nn_LNon_37460704756094 (embedding_lookup).

Math (reference):
    d   = (data - mean(data)) / std(data, ddof=1) * scalei
    s   = sigmoid(d); t = tanh(d)
    theta = interp(theta_lut, s * 119)   # theta_lut = linspace(-pi, pi, 120)
    velo  = interp(velo_lut, |t| * 119)  # velo_lut  = linspace(0, 3, 120)
    val = d * exp(velo * sin(theta)) + velo * cos(theta)
    out = (val - mean(val)) / std(val, ddof=1) * scaleo

Both LUTs are affine in the index, so interpolation collapses to affine maps
of sigmoid/|tanh|.  Using tanh(y/2) = 2*sigmoid(y) - 1, theta becomes an
affine map of u' = tanh(y/2), so the scalar-engine work per element is just
    u' = tanh(y/2), t = tanh(y), sn = sin(c1*u'+c2), cs = sin(c1*u'+c2+pi/2),
    E  = exp(v_slope * |t| * sn)
i.e. 5 activation passes, of which tanh/tanh/sin/sin share ONE activation
table set (silu_and_others) and exp a second -- minimizing ACT_TABLE_LOADs.
All element-wise arithmetic runs on the DVE in fp16 (2x/4x perf modes) with
reductions fused into accum_out, so no tensor_reduce passes over the data.

I/O is fp16 (host converts): halves HBM traffic vs fp32.

Distribution: batch-sharded over 8 cores (4 batches each = [128, 32768] f16
per core, SBUF-resident).  Global mean/std for both normalizations via
per-partition accumulation -> 8-core AllReduce of [128, 2] partials ->
ones-matmul partition-reduce+broadcast.  A dummy AllReduce and a dummy Sqrt
activation issue at t=0 to pre-warm the collective path and the sqrt table
under the input DMA.
"""

import math

import numpy as np

import concourse.bacc as bacc
import concourse.bass as bass
import concourse.mybir as mybir
import concourse.tile as tile
from concourse.bass_utils import run_bass_kernel_spmd

N_CORES = 8
P = 128
B_FULL, C, H, W = 32, 64, 128, 128
PER_CORE = B_FULL // N_CORES * C * H * W          # 4,194,304
FREE = PER_CORE // P                              # 32,768
CH = 8192                                         # A/C chunk + B group size
NCH = FREE // CH                                  # 4
N_TOTAL = B_FULL * C * H * W                      # 33,554,432

AF = mybir.ActivationFunctionType
ALU = mybir.AluOpType
AX = mybir.AxisListType
F32 = mybir.dt.float32
F16 = mybir.dt.float16
I16 = mybir.dt.int16

LAST_RESULT = None  # BassKernelResults of the most recent run (for test.py)

_KERNEL_CACHE = {}


def _build(consts, sim_mode=False):
    """Build the SPMD Bass program.

    `consts` = (sin_scale, sin_b1, v_slope): theta = th0 + th_slope*s =
    sin_scale*u' + sin_b1 with u' = tanh(y/2).
    """
    sin_scale, sin_b1, v_slope = consts
    halfpi = math.pi / 2.0
    # cos(theta) for theta = sin_scale*u' + sin_b1 with sin_b1 ~ 0 (symmetric
    # LUT): cos is even in u', so cos = sin(pi/2 - sin_scale*|u'| - sin_b1),
    # keeping the Sin argument inside the scalar engine's valid [-pi, pi].
    assert abs(sin_b1) < 1e-5, f"theta LUT must be symmetric (got b1={sin_b1})"
    sin_b2 = halfpi - sin_b1

    nc = bacc.Bacc(None, num_devices=N_CORES)

    # Float biases for Sin are looked up in nc.const_aps (keyed (F32, val)).
    for cv in (sin_b1, sin_b2):
        if (F32, cv) not in nc.const_aps.aps:
            t = nc.alloc_sbuf_tensor(f"const-f32-{cv}", [P, 1], F32)
            nc.gpsimd.memset(t.ap(), cv)
            nc.const_aps.aps[(F32, cv)] = t.ap()
    nc.all_engine_barrier()

    data_in = nc.dram_tensor("data", [P, FREE], F16, kind="ExternalInput")
    scal_in = nc.dram_tensor("scal", [P, 2], F32, kind="ExternalInput")
    out_dram = nc.dram_tensor("out", [P, FREE], F16, kind="ExternalOutput")

    groups = [list(range(N_CORES))]

    with tile.TileContext(nc) as tc:
        with (
            tc.tile_pool(name="big", bufs=1) as bigpool,
            tc.tile_pool(name="small", bufs=1) as smallpool,
            tc.tile_pool(name="psum", bufs=1, space="PSUM") as psumpool,
            tc.tile_pool(name="dram", bufs=1, space="DRAM") as dram,
        ):
            xb = bigpool.tile([P, FREE], F16, name="xb", tag="xb")
            # two ping-pong scratch sets for the paired B groups
            bufs = [
                [
                    bigpool.tile([P, CH], F16, name=f"s{s}b{i}", tag=f"s{s}b{i}")
                    for i in range(4)
                ]
                for s in range(2)
            ]
            # statbuf cols: [0:4) sum(x) per chunk, [4:8) sum(x^2) per chunk,
            # [8:12) sum(val) per group, [12:16) sum(val^2) per group
            statbuf = smallpool.tile([P, 16], F32, name="statbuf", tag="statbuf")
            sm = smallpool.tile([P, 32], F32, name="sm", tag="sm")
            stA = smallpool.tile([P, 2], F32, name="stA", tag="stA")
            stB = smallpool.tile([P, 2], F32, name="stB", tag="stB")
            scal_all = smallpool.tile([P, 2], F32, name="scal_all", tag="scal_all")
            ones = smallpool.tile([P, P], F32, name="ones", tag="ones")
            psumA = psumpool.tile([P, 2], F32, name="psumA", tag="psumA")
            psumB = psumpool.tile([P, 2], F32, name="psumB", tag="psumB")

            cc_w_in = dram.tile([P, 2], F32, name="cc_w_in", tag="cc_w_in")
            cc_w_out = dram.tile([P, 2], F32, name="cc_w_out", tag="cc_w_out")
            cc_a_in = dram.tile([P, 2], F32, name="cc_a_in", tag="cc_a_in")
            cc_a_out = dram.tile([P, 2], F32, name="cc_a_out", tag="cc_a_out")
            cc_b_in = dram.tile([P, 2], F32, name="cc_b_in", tag="cc_b_in")
            cc_b_out = dram.tile([P, 2], F32, name="cc_b_out", tag="cc_b_out")

            nc.gpsimd.dma_start(scal_all[:], scal_in[:])
            nc.vector.memset(ones[:], 1.0)

            # ---- warm-ups under the input DMA ----
            # (a) dummy AllReduce absorbs the collective cold-start so the
            # real stats AllReduce is cheap
            nc.vector.memset(stB[:], 0.0)
            nc.gpsimd.dma_start(cc_w_in[:], stB[:])
            if sim_mode:
                nc.gpsimd.dma_start(cc_w_out[:], cc_w_in[:])
            else:
                nc.gpsimd.collective_compute(
                    "AllReduce", ALU.add, replica_groups=groups,
                    ins=[cc_w_in.opt()], outs=[cc_w_out.opt()],
                )
            # (b) sqrt activation table (set also covers Square for phase A)
            nc.scalar.activation(sm[:, 30:31], ones[:, 0:1], AF.Sqrt)

            # ---------------- Phase A: load + input stats ----------------
            for c in range(NCH):
                sl = slice(c * CH, (c + 1) * CH)
                nc.sync.dma_start(xb[:, sl], data_in[:, sl])
                # sum(x^2): scalar Square (in every act table set) + accum
                nc.scalar.activation(
                    bufs[0][0][:], xb[:, sl], AF.Square,
                    accum_out=statbuf[:, 4 + c : 5 + c],
                )
                # sum(x): fp16 pairwise presum tree (tt runs 2x-mode; a raw
                # tensor_reduce over 8192 is 1x and much slower), then a
                # short 1x reduce. fp16 DVE accum_out faults the HW.
                TR = bufs[1][1]
                h = CH // 2
                nc.vector.tensor_tensor(
                    TR[:, 0:h], xb[:, c * CH : c * CH + h],
                    xb[:, c * CH + h : (c + 1) * CH], op=ALU.add)
                nc.vector.tensor_tensor(
                    TR[:, h : h + h // 2], TR[:, 0 : h // 2],
                    TR[:, h // 2 : h], op=ALU.add)
                nc.vector.reduce_sum(
                    statbuf[:, c : c + 1], TR[:, h : h + h // 2], axis=AX.X
                )

            nc.vector.reduce_sum(stA[:, 0:1], statbuf[:, 0:4], axis=AX.X)
            nc.vector.reduce_sum(stA[:, 1:2], statbuf[:, 4:8], axis=AX.X)

            nc.gpsimd.dma_start(cc_a_in[:], stA[:])
            if sim_mode:
                nc.gpsimd.dma_start(cc_a_out[:], cc_a_in[:])
            else:
                nc.gpsimd.collective_compute(
                    "AllReduce", ALU.add, replica_groups=groups,
                    ins=[cc_a_in.opt()], outs=[cc_a_out.opt()],
                )
            nc.gpsimd.dma_start(stA[:], cc_a_out[:])
            # ones.T @ stA reduces across partitions AND broadcasts
            nc.tensor.matmul(psumA[:], ones[:], stA[:])
            nc.vector.tensor_copy(sm[:, 0:2], psumA[:])

            # a = scalei / std, b = -mean * a   (std unbiased, ddof=1)
            nc.vector.tensor_scalar_mul(sm[:, 2:3], sm[:, 0:1], 1.0 / N_TOTAL)
            nc.vector.tensor_mul(sm[:, 3:4], sm[:, 0:1], sm[:, 2:3])
            nc.vector.tensor_sub(sm[:, 4:5], sm[:, 1:2], sm[:, 3:4])
            nc.vector.tensor_scalar_mul(sm[:, 5:6], sm[:, 4:5], 1.0 / (N_TOTAL - 1))
            nc.scalar.activation(sm[:, 6:7], sm[:, 5:6], AF.Sqrt)
            nc.vector.reciprocal(sm[:, 7:8], sm[:, 6:7])
            nc.vector.tensor_mul(sm[:, 8:9], sm[:, 7:8], scal_all[:, 0:1])   # a
            nc.vector.tensor_mul(sm[:, 9:10], sm[:, 2:3], sm[:, 8:9])
            nc.vector.tensor_scalar_mul(sm[:, 10:11], sm[:, 9:10], -1.0)     # b
            nc.vector.tensor_scalar_mul(sm[:, 11:12], sm[:, 8:9], 0.5)       # a/2
            nc.vector.tensor_scalar_mul(sm[:, 12:13], sm[:, 10:11], 0.5)     # b/2
            a_ap = sm[:, 8:9]
            b_ap = sm[:, 10:11]
            ah_ap = sm[:, 11:12]
            bh_ap = sm[:, 12:13]

            # ---------------- Phase B: 4 groups, paired over 2 buffer sets
            # Scalar passes per group: tanh(y/2), tanh(y), sin, sin [one
            # table set], then exp [second set].  Pairing groups keeps each
            # table set loaded across 2 groups -> 2 loads per pair.
            for pair in range(NCH // 2):
                gs = (2 * pair, 2 * pair + 1)
                for g in gs:
                    BU, BT, BSN, BCS = bufs[g % 2]
                    sl = slice(g * CH, (g + 1) * CH)
                    nc.scalar.activation(BU[:], xb[:, sl], AF.Tanh,
                                         bias=bh_ap, scale=ah_ap)       # u'
                    nc.scalar.activation(BT[:], xb[:, sl], AF.Tanh,
                                         bias=b_ap, scale=a_ap)         # t
                    nc.scalar.activation(BSN[:], BU[:], AF.Sin,
                                         bias=sin_b1, scale=sin_scale)  # sn
                for g in gs:
                    BU, BT, BSN, BCS = bufs[g % 2]
                    # |u'| in one 4x-mode op: clear the fp16 sign bit
                    nc.vector.tensor_scalar(
                        BCS[:].bitcast(I16), BU[:].bitcast(I16),
                        0x7FFF, None, op0=ALU.bitwise_and)             # |u'|
                for g in gs:
                    BU, BT, BSN, BCS = bufs[g % 2]
                    nc.scalar.activation(BU[:], BCS[:], AF.Sin,
                                         bias=sin_b2, scale=-sin_scale)  # cs
                for g in gs:
                    BU, BT, BSN, BCS = bufs[g % 2]
                    # rotate buffers so no op writes a tile it also reads
                    nc.vector.tensor_scalar(
                        BCS[:].bitcast(I16), BT[:].bitcast(I16),
                        0x7FFF, None, op0=ALU.bitwise_and)             # w = |t|
                    nc.vector.tensor_tensor(BT[:], BCS[:], BSN[:],
                                            op=ALU.mult)               # p''=w*sn
                    nc.vector.tensor_tensor(BSN[:], BCS[:], BU[:],
                                            op=ALU.mult)               # q''=w*cs
                for g in gs:
                    BU, BT, BSN, BCS = bufs[g % 2]
                    nc.scalar.activation(BCS[:], BT[:], AF.Exp,
                                         scale=v_slope)                 # E
                for g in gs:
                    BU, BT, BSN, BCS = bufs[g % 2]
                    sl = slice(g * CH, (g + 1) * CH)
                    nc.vector.tensor_scalar(
                        BU[:], xb[:, sl], a_ap, b_ap,
                        op0=ALU.mult, op1=ALU.add)                      # u = a*x+b
                    nc.vector.tensor_tensor(BT[:], BU[:], BCS[:],
                                            op=ALU.mult)                # r = u*E
                    nc.vector.tensor_scalar_mul(
                        BU[:], BSN[:], v_slope)                         # v*q''
                    nc.vector.tensor_tensor(xb[:, sl], BT[:], BU[:],
                                            op=ALU.add)                 # val
                    # stats via presum trees (see phase A comment)
                    h = CH // 2
                    q = CH // 4
                    nc.vector.tensor_tensor(BCS[:], xb[:, sl], xb[:, sl],
                                            op=ALU.mult)                # val^2
                    nc.vector.tensor_tensor(
                        BU[:, 0:h], BCS[:, 0:h], BCS[:, h:CH], op=ALU.add)
                    nc.vector.tensor_tensor(
                        BU[:, h : h + q], BU[:, 0:q], BU[:, q:h], op=ALU.add)
                    nc.vector.reduce_sum(
                        statbuf[:, 12 + g : 13 + g], BU[:, h : h + q],
                        axis=AX.X)                                      # sum val^2
                    nc.vector.tensor_tensor(
                        BT[:, 0:h], xb[:, g * CH : g * CH + h],
                        xb[:, g * CH + h : (g + 1) * CH], op=ALU.add)
                    nc.vector.tensor_tensor(
                        BT[:, h : h + q], BT[:, 0:q], BT[:, q:h], op=ALU.add)
                    nc.vector.reduce_sum(
                        statbuf[:, 8 + g : 9 + g], BT[:, h : h + q],
                        axis=AX.X)                                      # sum(val)

            nc.vector.reduce_sum(stB[:, 0:1], statbuf[:, 8:12], axis=AX.X)
            nc.vector.reduce_sum(stB[:, 1:2], statbuf[:, 12:16], axis=AX.X)

            nc.gpsimd.dma_start(cc_b_in[:], stB[:])
            if sim_mode:
                nc.gpsimd.dma_start(cc_b_out[:], cc_b_in[:])
            else:
                nc.gpsimd.collective_compute(
                    "AllReduce", ALU.add, replica_groups=groups,
                    ins=[cc_b_in.opt()], outs=[cc_b_out.opt()],
                )
            nc.gpsimd.dma_start(stB[:], cc_b_out[:])
            nc.tensor.matmul(psumB[:], ones[:], stB[:])
            nc.vector.tensor_copy(sm[:, 16:18], psumB[:])

            nc.vector.tensor_scalar_mul(sm[:, 18:19], sm[:, 16:17], 1.0 / N_TOTAL)
            nc.vector.tensor_mul(sm[:, 19:20], sm[:, 16:17], sm[:, 18:19])
            nc.vector.tensor_sub(sm[:, 20:21], sm[:, 17:18], sm[:, 19:20])
            nc.vector.tensor_scalar_mul(sm[:, 21:22], sm[:, 20:21], 1.0 / (N_TOTAL - 1))
            nc.scalar.activation(sm[:, 22:23], sm[:, 21:22], AF.Sqrt)
            nc.vector.reciprocal(sm[:, 23:24], sm[:, 22:23])
            nc.vector.tensor_mul(sm[:, 24:25], sm[:, 23:24], scal_all[:, 1:2])  # a2
            nc.vector.tensor_mul(sm[:, 25:26], sm[:, 18:19], sm[:, 24:25])
            nc.vector.tensor_scalar_mul(sm[:, 26:27], sm[:, 25:26], -1.0)       # b2
            a2_ap = sm[:, 24:25]
            b2_ap = sm[:, 26:27]

            # ---------------- Phase C: normalize + store -----------------
            for c in range(NCH):
                sl = slice(c * CH, (c + 1) * CH)
                o_ = bufs[c % 2][0]
                nc.vector.tensor_scalar(
                    o_[:], xb[:, sl], a2_ap, b2_ap, op0=ALU.mult, op1=ALU.add
                )
                nc.sync.dma_start(out_dram[:, sl], o_[:])

    nc.finalize()
    return nc


def kernel(data, params, scalei, scaleo):
    global LAST_RESULT
    params = np.asarray(params, dtype=np.float32)

    # Affine-LUT coefficients from the actual params input.
    th_lut = params[0, 0]
    v_lut = params[1, 0]
    npts = th_lut.shape[0]
    th0 = float(th_lut[0])
    th_slope = float(th_lut[npts - 1]) - th0
    v0 = float(v_lut[0])
    v_slope = float(v_lut[npts - 1]) - v0
    assert abs(v0) < 1e-6, f"velocity LUT must start at 0 (got {v0})"

    # theta = th0 + th_slope*s = sin_scale*u' + sin_b1, u' = tanh(y/2)
    consts = (th_slope / 2.0, th0 + th_slope / 2.0, v_slope)
    nc = _KERNEL_CACHE.get(consts)
    if nc is None:
        nc = _build(consts)
        _KERNEL_CACHE[consts] = nc

    scal = np.tile(
        np.array(
            [[float(np.asarray(scalei).reshape(-1)[0]),
              float(np.asarray(scaleo).reshape(-1)[0])]],
            dtype=np.float32,
        ),
        (P, 1),
    )

    data = np.asarray(data)
    bpc = B_FULL // N_CORES
    in_maps = []
    for i in range(N_CORES):
        shard = (
            data[i * bpc : (i + 1) * bpc]
            .reshape(P, FREE)
            .astype(np.float16)
        )
        in_maps.append({"data": shard, "scal": scal})

    res = run_bass_kernel_spmd(nc, in_maps, core_ids=list(range(N_CORES)))
    LAST_RESULT = res

    out = np.concatenate(
        [
            r["out"].astype(np.float32).reshape(bpc, C, H, W)
            for r in res.results
        ],
        axis=0,
    )
    return out
